# revision 11
# baseline (speedup 1.0000x reference)
"""Bass/Trainium2 kernel for nn_Attention_66297115181568 (sparse_attention).

Strategy: head-parallel across 8 NeuronCores; core h computes head h
end-to-end and its 64-row slice of the Wo projection. The host sums the
8 partial (512, 512) outputs (the tensor-parallel all-reduce) and adds bo.

Key optimizations over the v1 kernel (246us):
  1. rel_pos is cast to fp8e4 (x64 scale, folded back via the block-diag
     mask constant) on the host: 16MB/core HBM traffic instead of 64MB.
  2. The q.rel contraction runs on the TensorEngine via a block-diagonal
     stationary: for each 32-row block b and d-group g (4 d's), stationary
     Qrow_g[:, 32b:32b+32] is q' values scattered block-diagonally so
     out[i, j] += sum_d q'[i, d] rel[i, j, d]. 4 blocks run concurrently
     in distinct PE column groups (tile_position), accumulating into one
     PSUM bank that also receives the QK^T logits and the mask row, so
     softmax reads a single PSUM tile.
  3. Projections are computed in transposed form (qT/kT/vT) with the
     rotate-half of RoPE folded into host-rotated weight copies (q_rot =
     x @ (W @ R)), eliminating all x/q/k transposes on the PE.
  4. All matmuls in bf16/fp8 (f32 PSUM accumulation); softmax kept f32.
  5. rel stream issued as 2MB DMAs alternating across both HWDGE rings.
"""

import sys

sys.path.insert(0, "/opt/trn_rl_repo")

from contextlib import ExitStack

import numpy as np
import ml_dtypes

import concourse.bass as bass
import concourse.tile as tile
from concourse import mybir

# problem dims (hardcoded per spec)
B, N, DIM, H, D = 1, 512, 512, 8, 64
INNER = H * D
N_CORES = 8
P = 128                 # SBUF partitions
NT = N // P             # 4 row tiles
KT = DIM // P           # 4 contraction tiles for projections
IB = 32                 # i-block rows (PE col-group granularity)
NB = N // IB            # 16 blocks
DG = 4                  # d's per d-group
NG = D // DG            # 16 d-groups
SCALE = D ** -0.5
NEG_BIG = 3.0e38
RELSCALE = 64.0         # host scales rel by this before fp8 cast

f32 = mybir.dt.float32
bf16 = mybir.dt.bfloat16
fp8 = mybir.dt.float8e4
AX = mybir.AxisListType
ALU = mybir.AluOpType
AF = mybir.ActivationFunctionType


def legalize_multi_waits(nc):
    """This walrus build supports only one sync-wait per instruction; hoist
    extra waits onto same-engine NoOps placed immediately before."""
    nid = 0
    for fn in nc.m.functions:
        for bb in fn.blocks:
            new = []
            changed = False
            for inst in bb.instructions:
                si = inst.sync_info
                waits = si.on_wait if si is not None else []
                if len(waits) > 1:
                    for w in waits[:-1]:
                        nop = mybir.InstNoOp(name=f"I-waitfix-{nid}")
                        nid += 1
                        nop.engine = inst.engine
                        nop.sync_info = mybir.SyncInfo(on_wait=[w], on_update=[])
                        new.append(nop)
                    si.on_wait = [waits[-1]]
                    inst.sync_info = si
                    changed = True
                new.append(inst)
            if changed:
                bb.instructions = new


def build_nc():
    nc = bass.Bass()

    xt_ext = nc.declare_dram_parameter("xt", [DIM, N], bf16, isOutput=False)
    # q/k/qrot/krot weight slices, [DIM, D] each, packed [DIM, 4*D]
    wqks_ext = nc.declare_dram_parameter("wqks", [DIM, 4 * D], bf16, isOutput=False)
    wv_ext = nc.declare_dram_parameter("wv", [DIM, D], bf16, isOutput=False)
    bqks_ext = nc.declare_dram_parameter("bqks", [1, 4 * D], f32, isOutput=False)
    bv_ext = nc.declare_dram_parameter("bv", [1, D], f32, isOutput=False)
    wo_ext = nc.declare_dram_parameter("wo", [D, DIM], bf16, isOutput=False)
    # cos/sin in transposed layout, [D, N] each: rows for q (plain) and for
    # k (scaled by D**-0.5), packed [4*D rows: cosq, cosk, sinq, sink] -- but
    # loaded as a [D, 4, N] tile (partition dim D).
    cs_ext = nc.declare_dram_parameter("cs", [D, 4 * N], bf16, isOutput=False)
    tconst_ext = nc.declare_dram_parameter("tconst", [D, NG * P], bf16, isOutput=False)
    m512_ext = nc.declare_dram_parameter("m512", [P, N], bf16, isOutput=False)
    maskrow_ext = nc.declare_dram_parameter("maskrow", [1, N], f32, isOutput=False)
    identf_ext = nc.declare_dram_parameter("identf", [P, P], f32, isOutput=False)
    identb_ext = nc.declare_dram_parameter("identb", [P, P], bf16, isOutput=False)
    # rel blocks: [pair, p, (b2, g, j)] fp8, one 2MB row per pair of 32-i blocks
    rel_ext = nc.declare_dram_parameter("rel", [NB // 2, P, 2 * NG * N], fp8,
                                        isOutput=False)
    out_ext = nc.declare_dram_parameter("out", [N, DIM], f32, isOutput=True)

    with tile.TileContext(nc) as tc, ExitStack() as ctx:
        dma = nc.sync      # HWDGE ring 1: even rel pairs
        dma2 = nc.scalar   # HWDGE ring 2: inputs, odd rel pairs, outputs
        consts = ctx.enter_context(tc.tile_pool(name="consts", bufs=1))
        pro = ctx.enter_context(tc.tile_pool(name="pro", bufs=1))
        relp = ctx.enter_context(tc.tile_pool(name="relp", bufs=4))
        smp = ctx.enter_context(tc.tile_pool(name="smp", bufs=2))
        smallp = ctx.enter_context(tc.tile_pool(name="smallp", bufs=2))
        outp = ctx.enter_context(tc.tile_pool(name="outp", bufs=2))
        # PSUM: psA = proj (prologue) + dots (main); psB = rep (prologue) +
        # out (main); psW = w/v transposes; psV = attnT accumulation
        psA = ctx.enter_context(
            tc.tile_pool(name="psA", bufs=2, space=bass.MemorySpace.PSUM))
        psB = ctx.enter_context(
            tc.tile_pool(name="psB", bufs=2, space=bass.MemorySpace.PSUM))
        psW = ctx.enter_context(
            tc.tile_pool(name="psW", bufs=2, space=bass.MemorySpace.PSUM))
        psV = ctx.enter_context(
            tc.tile_pool(name="psV", bufs=1, space=bass.MemorySpace.PSUM))
        psO = ctx.enter_context(
            tc.tile_pool(name="psO", bufs=1, space=bass.MemorySpace.PSUM))

        # ---- small inputs on ring 2, projection dependencies first ----
        xt_sb = pro.tile([P, KT, N], bf16)
        dma2.dma_start(out=xt_sb[:], in_=xt_ext.rearrange("(u p) n -> p u n", p=P))
        wqks_sb = pro.tile([P, KT, 4 * D], bf16)
        dma2.dma_start(out=wqks_sb[:],
                       in_=wqks_ext.rearrange("(u p) m -> p u m", p=P))
        wv_sb = pro.tile([P, KT, D], bf16)
        dma2.dma_start(out=wv_sb[:], in_=wv_ext.rearrange("(u p) m -> p u m", p=P))
        bqks_sb = consts.tile([1, 4 * D], f32)
        dma2.dma_start(out=bqks_sb[:], in_=bqks_ext[:])
        bv_sb = consts.tile([1, D], f32)
        dma2.dma_start(out=bv_sb[:], in_=bv_ext[:])
        wo_sb = consts.tile([D, DIM], bf16)
        dma2.dma_start(out=wo_sb[:], in_=wo_ext[:])
        cs_sb = consts.tile([D, 4, N], bf16)
        dma2.dma_start(out=cs_sb[:], in_=cs_ext.rearrange("d (c n) -> d c n", c=4))
        tconst_sb = consts.tile([D, NG, P], bf16)
        dma2.dma_start(out=tconst_sb[:],
                       in_=tconst_ext.rearrange("d (g p) -> d g p", g=NG))
        m512_sb = consts.tile([P, N], bf16)
        dma2.dma_start(out=m512_sb[:], in_=m512_ext[:])
        identf = consts.tile([P, P], f32)
        dma2.dma_start(out=identf[:], in_=identf_ext[:])
        identb = consts.tile([P, P], bf16)
        dma2.dma_start(out=identb[:], in_=identb_ext[:])
        maskrow_sb = consts.tile([1, N], f32)
        dma2.dma_start(out=maskrow_sb[:], in_=maskrow_ext[:])
        ones_sb = consts.tile([1, N], f32)
        nc.vector.memset(ones_sb, 1.0)

        # ---- rel stream: 8 x 2MB, alternating rings ----
        rel_tiles = []
        for pr_ in range(NB // 2):
            rl = relp.tile([P, 2, NG, N], fp8)
            eng = dma if pr_ % 2 == 0 else dma2
            eng.dma_start(out=rl[:],
                          in_=rel_ext[pr_].rearrange("p (b g j) -> p b g j",
                                                     b=2, g=NG))
            rel_tiles.append(rl)

        # ---- projections (transposed): qT, kT, qrotT, krotT, vT ----
        qkT_sb = pro.tile([D, 4, N], bf16)
        for c in range(4):
            ps_c = psA.tile([P, N], f32, tag="big")
            for u in range(KT):
                nc.tensor.matmul(ps_c[0:D, :],
                                 wqks_sb[:, u, c * D:(c + 1) * D],
                                 xt_sb[:, u, :], start=(u == 0), stop=False)
            nc.tensor.matmul(ps_c[0:D, :], bqks_sb[:, c * D:(c + 1) * D],
                             ones_sb[:], start=False, stop=True)
            nc.scalar.copy(qkT_sb[:, c, :], ps_c[0:D, :])

        ps_v = psA.tile([P, N], f32, tag="big")
        for u in range(KT):
            nc.tensor.matmul(ps_v[0:D, :], wv_sb[:, u, :], xt_sb[:, u, :],
                             start=(u == 0), stop=False)
        nc.tensor.matmul(ps_v[0:D, :], bv_sb[:], ones_sb[:], start=False, stop=True)
        vT_sb = pro.tile([D, N], f32)
        nc.scalar.copy(vT_sb[:], ps_v[0:D, :])

        # ---- RoPE on DVE: q'T = cosq*qT + sinq*qrotT; k' likewise (x SCALE) ----
        qkp_sb = pro.tile([D, 2, N], bf16)
        t1 = pro.tile([D, N], bf16, tag="ropet1")
        t2 = pro.tile([D, N], bf16, tag="ropet2")
        for c in range(2):  # 0: q, 1: k
            nc.vector.tensor_mul(t1[:], qkT_sb[:, c, :], cs_sb[:, c, :])
            nc.vector.tensor_mul(t2[:], qkT_sb[:, 2 + c, :], cs_sb[:, 2 + c, :])
            nc.vector.tensor_add(qkp_sb[:, c, :], t1[:], t2[:])
        qpT = qkp_sb[:, 0, :]
        kpT = qkp_sb[:, 1, :]

        # ---- v -> [j, d] layout via PE transposes ----
        v_sb = pro.tile([P, NT, D], bf16)
        for jt in range(NT):
            pv = psW.tile([P, P], f32, tag="tp")
            nc.tensor.transpose(pv[:, 0:D], vT_sb[:, jt * P:(jt + 1) * P],
                                identf[0:D, 0:D])
            nc.scalar.copy(v_sb[:, jt, :], pv[:, 0:D])

        # ---- Qrow: replicate q'T across partitions per d-group, then mask ----
        # Rep_g[p, n] = q'T[g*4 + p%4, n];  Qrow_g = Rep_g * m512 (bf16)
        # m512 carries the block-diagonal delta and the 1/RELSCALE factor.
        # One tile per g so rel matmuls depend only on their own g; the repl
        # matmuls interleave into tile 0's rel stream with one-g lookahead.
        qrow_tiles = [consts.tile([P, N], bf16, name=f"qrow{g}")
                      for g in range(NG)]

        def emit_repl(g):
            ps_rep = psB.tile([P, N], f32, tag="rep")
            nc.tensor.matmul(ps_rep[:], tconst_sb[:, g, :], qpT,
                             start=True, stop=True)
            nc.vector.tensor_mul(qrow_tiles[g][:], ps_rep[:], m512_sb[:])

        emit_repl(0)

        # ---- main loop over row tiles ----
        for it in range(NT):
            dots_ps = psA.tile([P, N], f32, tag="big")
            # rel-term: 4 blocks concurrently in distinct PE column groups
            for g in range(NG):
                if it == 0 and g + 1 < NG:
                    emit_repl(g + 1)
                for bl in range(NT):
                    b = it * NT + bl
                    rl = rel_tiles[b // 2]
                    nc.tensor.matmul(
                        dots_ps[bl * IB:(bl + 1) * IB, :],
                        qrow_tiles[g][:, b * IB:(b + 1) * IB],
                        rl[:, b % 2, g, :],
                        start=(g == 0), stop=False,
                        tile_position=(0, bl * IB))
            # QK^T logits accumulate into the same PSUM bank
            nc.tensor.matmul(dots_ps[:], qpT[:, it * P:(it + 1) * P], kpT,
                             start=False, stop=False)
            # mask bias row (additive, 0 kept / -BIG masked)
            nc.tensor.matmul(dots_ps[:], ones_sb[:, 0:P], maskrow_sb[:],
                             start=False, stop=True, skip_group_check=True)

            # softmax (unnormalized; 1/rowsum folded into the output copy)
            negmax = smallp.tile([P, 1], f32, tag="negmax")
            nc.vector.tensor_reduce(negmax[:], dots_ps[:], AX.X, ALU.max,
                                    negate=True)
            w_sm = smp.tile([P, N], f32, tag="w_sm")
            rowsum = smallp.tile([P, 1], f32, tag="rowsum")
            nc.scalar.activation(w_sm[:], dots_ps[:], AF.Exp, bias=negmax[:],
                                 accum_out=rowsum[:])
            rcp = smallp.tile([P, 1], f32, tag="rcp")
            nc.vector.reciprocal(rcp[:], rowsum[:])

            wT_sb = outp.tile([P, NT, P], bf16, tag="wT_sb")
            for jt in range(NT):
                wp = psW.tile([P, P], f32, tag="tp")
                nc.tensor.transpose(wp[:], w_sm[:, jt * P:(jt + 1) * P], identf[:])
                nc.scalar.copy(wT_sb[:, jt, :], wp[:])

            attn_ps = psV.tile([D, P], f32, tag="attn")
            for jt in range(NT):
                nc.tensor.matmul(attn_ps[:], v_sb[:, jt, :], wT_sb[:, jt, :],
                                 start=(jt == 0), stop=(jt == NT - 1))
            attn_sb = outp.tile([D, P], bf16, tag="attn_sb")
            nc.scalar.copy(attn_sb[:], attn_ps[:])

            out_ps = psO.tile([P, DIM], f32, tag="out")
            nc.tensor.matmul(out_ps[:], attn_sb[:], wo_sb[:], start=True, stop=True)
            o_sb = outp.tile([P, DIM], f32, tag="o_sb")
            nc.scalar.activation(o_sb[:], out_ps[:], AF.Copy, scale=rcp[:])
            nc.gpsimd.dma_start(out=out_ext[it * P:(it + 1) * P, :], in_=o_sb[:])

    legalize_multi_waits(nc)
    return nc


_NC_CACHE = None
TRACE = False        # set by test harness to capture an NTFF profile
LAST_RESULT = None   # BassKernelResults of the most recent kernel() call


def _get_nc():
    global _NC_CACHE
    if _NC_CACHE is None:
        _NC_CACHE = build_nc()
    return _NC_CACHE


def _rot_mat():
    """rotate_half as a right-multiply matrix: rot(q) = q @ Rm."""
    Rm = np.zeros((D, D), np.float32)
    for i in range(D // 2):
        Rm[2 * i + 1, 2 * i] = -1.0
        Rm[2 * i, 2 * i + 1] = 1.0
    return Rm


def kernel(**inputs):
    x = np.asarray(inputs["x"], dtype=np.float32)
    mask = np.asarray(inputs["mask"])
    rope = np.asarray(inputs["rope"], dtype=np.float32)
    rel_pos = np.asarray(inputs["rel_pos"], dtype=np.float32)
    Wq = np.asarray(inputs["Wq"], dtype=np.float32)
    bq = np.asarray(inputs["bq"], dtype=np.float32)
    Wk = np.asarray(inputs["Wk"], dtype=np.float32)
    bk = np.asarray(inputs["bk"], dtype=np.float32)
    Wv = np.asarray(inputs["Wv"], dtype=np.float32)
    bv = np.asarray(inputs["bv"], dtype=np.float32)
    Wo = np.asarray(inputs["Wo"], dtype=np.float32)
    bo = np.asarray(inputs["bo"], dtype=np.float32)

    nc = _get_nc()
    Rm = _rot_mat()

    xT = np.ascontiguousarray(x.reshape(N, DIM).T).astype(ml_dtypes.bfloat16)
    maskrow = ((mask.reshape(1, N).astype(np.float32)) - 1.0) * NEG_BIG

    # cos/sin in transposed layout, packed [D, 4*N]: cosq, cosk, sinq, sink
    # (k columns carry the QK scale, folded here)
    cosT = np.cos(rope).T.astype(np.float32)      # [D, N]
    sinT = np.sin(rope).T.astype(np.float32)
    cs = np.concatenate([cosT, cosT * SCALE, sinT, sinT * SCALE],
                        axis=1).astype(ml_dtypes.bfloat16)

    # T[d, g, p] = (d == g*4 + p%4); m512[p, n] = (n%32 == p//4)/RELSCALE
    d_i = np.arange(D)[:, None, None]
    g_i = np.arange(NG)[None, :, None]
    p_i = np.arange(P)[None, None, :]
    tconst = (d_i == g_i * DG + p_i % DG).astype(np.float32)
    tconst = tconst.reshape(D, NG * P).astype(ml_dtypes.bfloat16)
    p_2 = np.arange(P)[:, None]
    n_2 = np.arange(N)[None, :]
    m512 = (((n_2 % IB) == (p_2 // DG)).astype(np.float32) / RELSCALE)
    m512 = m512.astype(ml_dtypes.bfloat16)

    identf = np.eye(P, dtype=np.float32)
    identb = identf.astype(ml_dtypes.bfloat16)

    # rel blocks: [h, pair, p=(i_l*4+d_l), (b2, g, j)] fp8, scaled by RELSCALE
    rel8 = (rel_pos[0] * RELSCALE).astype(ml_dtypes.float8_e4m3)
    # [h, (pair, b2, i_l), j, (g, d_l)] -> [h, pair, i_l, d_l, b2, g, j]
    rel8 = rel8.reshape(H, NB // 2, 2, IB, N, NG, DG)
    rel8 = np.ascontiguousarray(rel8.transpose(0, 1, 3, 6, 2, 5, 4))
    rel8 = rel8.reshape(H, NB // 2, P, 2 * NG * N)

    in_maps = []
    for h in range(N_CORES):
        sl = slice(h * D, (h + 1) * D)
        wq, wk = Wq[:, sl], Wk[:, sl]
        wqks = np.concatenate([wq, wk, wq @ Rm, wk @ Rm], axis=1)
        bqks = np.concatenate([bq[sl], bk[sl], bq[sl] @ Rm, bk[sl] @ Rm])[None, :]
        in_maps.append({
            "xt": xT,
            "wqks": np.ascontiguousarray(wqks).astype(ml_dtypes.bfloat16),
            "wv": np.ascontiguousarray(Wv[:, sl]).astype(ml_dtypes.bfloat16),
            "bqks": np.ascontiguousarray(bqks.astype(np.float32)),
            "bv": np.ascontiguousarray(bv[sl][None, :]),
            "wo": np.ascontiguousarray(Wo[sl, :]).astype(ml_dtypes.bfloat16),
            "cs": cs,
            "tconst": tconst,
            "m512": m512,
            "maskrow": maskrow,
            "identf": identf,
            "identb": identb,
            "rel": rel8[h],
        })

    from concourse.bass_utils import run_bass_kernel_spmd
    res = run_bass_kernel_spmd(nc, in_maps, list(range(N_CORES)), trace=TRACE)
    globals()["LAST_RESULT"] = res
    out = np.zeros((N, DIM), dtype=np.float32)
    for h in range(N_CORES):
        out += res.results[h]["out"]
    out += bo[None, :]
    return out.reshape(B, N, DIM)
